# revision 50
# baseline (speedup 1.0000x reference)
"""Trainium2 Bass kernel for multi-head self-attention.

Problem: B=4, S=2048, D=512, H=8 heads (DK=64), no mask, softmax without
max-subtraction (faithful to reference): attn = exp(s) / (sum(exp(s)) + 1e-8).

Sharding over 8 cores: core c handles batch b = c // 2 and the 4 heads
h0 = 4*(c % 2) .. h0+4 (x sharded by batch, weights column-sharded by head).

Per-core device pipeline (all matmuls in bf16, fp32 accumulate):
  1. Load x_b [2048, 512] fp32, PE-transpose to xT [d, s], cast bf16.
  2. Project qT/kT [e, s] (e on partitions) and v [s, e] (natural, augmented
     with a ones column per head so the PV matmul also produces the softmax
     denominator row).
  3. Per (head, q-half): loop k-tiles: scoresT = kT.T @ qT in PSUM (fp32),
     ACT exp (scale=1/8 folded in) -> SBUF bf16, PV matmul accumulates
     ctx_aug [65, 1024] in PSUM (row 64 = denominator).
  4. Finalize: PE-transpose ctx back to [q, e], multiply by
     1/(denom + 1e-8), stage in SBUF, DMA out.
"""

import os
import sys
from contextlib import ExitStack

import numpy as np

# concourse ships with the container; make sure it is importable even if
# the caller's PYTHONPATH doesn't include the repo.
for _p in ("/opt/trn_rl_repo", "/opt/pypackages"):
    if os.path.isdir(_p) and _p not in sys.path:
        sys.path.append(_p)

import concourse.bass as bass
import concourse.tile as tile
from concourse import bacc, mybir
from concourse.bass_utils import run_bass_kernel_spmd
from concourse.masks import make_identity

F32 = mybir.dt.float32
BF16 = mybir.dt.bfloat16

B, S, D, H = 4, 2048, 512, 8
DK = D // H
SCALE = 1.0 / np.sqrt(DK)
N_CORES = 8
P = 128

HPC = H // 2          # heads per core = 4
E = HPC * DK          # per-core output width = 256
NS = S // P           # 16 s-tiles
NDC = D // P          # 4 d-chunks
NEC = E // P          # 2 e-chunks of projected heads
QH = 512              # q processed per attention block
NQH = S // QH         # 4
EA = DK + 1           # 65: head context + denominator row


def _build_kernel(ctx: ExitStack, nc: bass.Bass, tc: tile.TileContext):
    xt = nc.dram_tensor("xt", [D, S], F32, kind="ExternalInput").ap()
    wq = nc.dram_tensor("wq", [D, E], F32, kind="ExternalInput").ap()
    wk = nc.dram_tensor("wk", [D, E], F32, kind="ExternalInput").ap()
    wv = nc.dram_tensor("wv", [D, E], F32, kind="ExternalInput").ap()
    out = nc.dram_tensor("out", [S, E], F32, kind="ExternalOutput").ap()

    const = ctx.enter_context(tc.tile_pool(name="const", bufs=1))
    xstage = ctx.enter_context(tc.tile_pool(name="xstage", bufs=3))
    persist = ctx.enter_context(tc.tile_pool(name="persist", bufs=1))
    exps = ctx.enter_context(tc.tile_pool(name="exps", bufs=4))
    fin = ctx.enter_context(tc.tile_pool(name="fin", bufs=2))
    # PSUM budget (8 banks): "sc" slots 2 x 2 banks (f32 [128,1024] pair
    # scores, also proj/transpose staging) + "ctx" slots 4 x 1 bank
    # ([65,512] f32 PV accumulators, finalize transposes) = 8 banks.
    ps_big = ctx.enter_context(tc.tile_pool(name="ps_big", bufs=2, space="PSUM"))
    ps_ctx = ctx.enter_context(tc.tile_pool(name="ps_ctx", bufs=4, space="PSUM"))

    ident = const.tile([P, P], F32)
    make_identity(nc, ident)
    ident_bf = const.tile([P, P], BF16)
    nc.vector.tensor_copy(out=ident_bf[:], in_=ident[:])

    # ---- PE warmup: dense fp32 matmuls during the x DMA window so HAM
    # un-throttles (K=8/8) before the projection matmuls arrive ----
    # sized to span the whole x-chunk-0 DMA window: if the PE idles >3.4us
    # after the warmup, HAM re-throttles and the projections run at 1.2GHz
    warm = ps_big.tile([P, P], F32, tag="sc", name="warm")
    for _ in range(12):
        nc.tensor.matmul(warm[:], lhsT=ident, rhs=ident, start=True, stop=True)

    # ---- Phases A+B: x chunk 0 first (longest pole), weights threaded in,
    # then the remaining x chunks. x is transposed on the host; the DMA
    # still moves the full fp32 x. Cast everything to bf16 on DVE. ----
    xt3 = xt.rearrange("(dc p) s -> p dc s", p=P)
    xT = persist.tile([P, NDC, S], BF16, name="xT")
    w_bf = {}

    def load_x_chunk(sc, split=False):
        xs = xstage.tile([P, NDC, QH], F32, tag="xs", name="xs")
        if split:  # two DMA queues (HWDGE + SWDGE) halve the landing time
            nc.sync.dma_start(
                xs[:, :, 0 : QH // 2], xt3[:, :, sc * QH : sc * QH + QH // 2]
            )
            nc.gpsimd.dma_start(
                xs[:, :, QH // 2 : QH], xt3[:, :, sc * QH + QH // 2 : (sc + 1) * QH]
            )
        else:
            nc.sync.dma_start(xs[:], xt3[:, :, sc * QH : (sc + 1) * QH])
        nc.vector.tensor_copy(out=xT[:, :, sc * QH : (sc + 1) * QH], in_=xs[:])

    def load_w(name, wap):
        wf = xstage.tile([P, NDC, E], F32, tag="wstage", name=f"{name}f")
        nc.sync.dma_start(wf[:], wap.rearrange("(dc p) e -> p dc e", p=P))
        wb = persist.tile([P, NDC, E], BF16, tag=f"{name}b", name=f"{name}b")
        nc.vector.tensor_copy(out=wb[:], in_=wf[:])
        w_bf[name] = wb

    load_x_chunk(0)
    load_w("wk", wk)
    load_w("wq", wq)
    load_w("wv", wv)
    load_x_chunk(1)
    load_x_chunk(2)
    load_x_chunk(3)

    # qT/kT: [e, s] with e on partitions (2 chunks of 128 = 4 heads)
    qT = persist.tile([P, NEC, S], BF16, name="qT")
    kT = persist.tile([P, NEC, S], BF16, name="kT")
    # v natural [s, e] in ones-augmented per-head layout [128, h, 65]
    v_aug = persist.tile([P, NS, HPC, EA], BF16, name="v_aug")
    nc.gpsimd.memset(v_aug[:, :, :, DK], 1.0)
    out_sb = persist.tile([P, NS, E], F32, name="out_sb")

    def proj_qk(dst, wname, ec, sc, tag, s0=0, sw=512):
        """dst[:, ec, sc*512+s0 : +sw] = (W row chunk).T @ xT chunk (bf16)."""
        wb = w_bf[wname]
        pp = ps_big.tile([P, 512], F32, tag=tag, name="pp") if tag == "sc" else \
            ps_ctx.tile([P, 512], F32, tag=tag, name="pp")
        lo = sc * 512 + s0
        for dc in range(NDC):
            nc.tensor.matmul(
                pp[:, 0:sw],
                lhsT=wb[:, dc, ec * P : (ec + 1) * P],
                rhs=xT[:, dc, lo : lo + sw],
                start=(dc == 0),
                stop=(dc == NDC - 1),
            )
        nc.vector.tensor_copy(out=dst[:, ec, lo : lo + sw], in_=pp[:, 0:sw])

    def proj_v(st, tag):
        vp = ps_big.tile([P, E], F32, tag=tag, name="vp") if tag == "sc" else \
            ps_ctx.tile([P, E], F32, tag=tag, name="vp")
        wvb = w_bf["wv"]
        for dc in range(NDC):
            nc.tensor.matmul(
                vp[:],
                lhsT=xT[:, dc, st * P : (st + 1) * P],
                rhs=wvb[:, dc, :],
                start=(dc == 0),
                stop=(dc == NDC - 1),
            )
        nc.vector.tensor_copy(
            out=v_aug[:, st, :, 0:DK],
            in_=vp.rearrange("p (h e) -> p h e", e=DK),
        )

    def attn_block(ec, qh, mid_work=None, pre_pv_work=None):
        """One (head pair, q-chunk) attention block. mid_work / pre_pv_work
        map kt -> thunks emitted inside the k loop (after PV / between exp
        and PV) so projections hide in the PE slack of the ACT stream."""
        ctx_a = ps_ctx.tile([EA, QH], F32, tag="ctx", name="ctx_a")
        ctx_b = ps_ctx.tile([EA, QH], F32, tag="ctx", name="ctx_b")
        for kt in range(NS):
            # scores for both heads, f32 psum [128, 2*QH]; the two matmuls
            # occupy PE row groups 0-63 / 64-127 concurrently
            sc_ps = ps_big.tile([P, 2 * QH], F32, tag="sc", name="sc_ps")
            for hb in range(2):
                nc.tensor.matmul(
                    sc_ps[:, hb * QH : (hb + 1) * QH],
                    lhsT=kT[hb * DK : (hb + 1) * DK, ec, kt * P : (kt + 1) * P],
                    rhs=qT[hb * DK : (hb + 1) * DK, ec, qh * QH : (qh + 1) * QH],
                    start=True,
                    stop=True,
                )
            ex = exps.tile([P, 2 * QH], BF16, tag="ex", name="ex")
            nc.scalar.activation(
                ex[:], sc_ps[:], mybir.ActivationFunctionType.Exp, scale=SCALE
            )
            if pre_pv_work and kt in pre_pv_work:
                for thunk in pre_pv_work[kt]:
                    thunk()
            for hb, ctx_ps in ((0, ctx_a), (1, ctx_b)):
                nc.tensor.matmul(
                    ctx_ps[:],
                    lhsT=v_aug[:, kt, 2 * ec + hb, :],
                    rhs=ex[:, hb * QH : (hb + 1) * QH],
                    start=(kt == 0),
                    stop=(kt == NS - 1),
                )
            if mid_work and kt in mid_work:
                for thunk in mid_work[kt]:
                    thunk()

        # finalize, split per head so the two halves land in different kts
        # of the next block: transpose ctx back to [q, e] (bf16 input
        # halves the PE cost), normalize into a shared staging tile, DMA
        # out after the second head
        po_cell = []

        def fin_head(hb, ctx_ps):
            if not po_cell:
                po_cell.append(fin.tile([P, QH // P, P], F32, tag="po", name="po"))
            po = po_cell[0]
            caug = fin.tile([EA, QH], BF16, tag="caug", name="caug")
            nc.vector.tensor_copy(out=caug[:], in_=ctx_ps[:])
            pt = ps_ctx.tile([P, 4, EA + 1], BF16, tag="ctx", name="pt")
            for j in range(QH // P):
                nc.tensor.transpose(
                    pt[:, j, 0:EA],
                    caug[:, j * P : (j + 1) * P],
                    ident_bf[0:EA, 0:EA],
                )
            den = fin.tile([P, 4], F32, tag="den", name="den")
            nc.vector.tensor_scalar_add(den[:], pt[:, :, DK], 1e-8)
            rec = fin.tile([P, 4], F32, tag="rec", name="rec")
            nc.vector.reciprocal(rec[:], den[:])
            for j in range(QH // P):
                nc.vector.tensor_scalar_mul(
                    po[:, j, hb * DK : (hb + 1) * DK],
                    pt[:, j, 0:DK],
                    rec[:, j : j + 1],
                )
            if hb == 1:
                nc.sync.dma_start(
                    out[qh * QH : (qh + 1) * QH, ec * P : (ec + 1) * P].rearrange(
                        "(t p) e -> p t e", p=P
                    ),
                    po[:],
                )

        return (lambda: fin_head(0, ctx_a), lambda: fin_head(1, ctx_b))

    # ---- Phases C+D interleaved: only block 0's immediate dependencies
    # are projected up front; everything else is paced into the k loops of
    # the 8 attention blocks to hide in PE slack under the ACT stream ----
    # first scores matmul only reads kT columns 0-127, so project those
    # first to start the exp stream sooner; the rest follows immediately
    proj_qk(kT, "wk", 0, 0, "sc", s0=0, sw=P)
    proj_qk(qT, "wq", 0, 0, "sc")
    proj_qk(kT, "wk", 0, 0, "sc", s0=P, sw=512 - P)

    mids = [dict() for _ in range(2 * NQH)]
    pres = [dict() for _ in range(2 * NQH)]

    def _add(d, b, kt, thunk):
        d[b].setdefault(kt, []).append(thunk)

    # v projections: v[kt] is consumed by block 0's PV at kt, so emit
    # proj_v(kt+1) between exp(kt) and PV(kt); v0 also goes there (off the
    # first-scores critical path -- it would otherwise gate the exp stream
    # on the wv DMA)
    _add(pres, 0, 0, lambda: proj_v(0, "ctx"))
    for st in range(1, NS):
        _add(pres, 0, st - 1, lambda st=st: proj_v(st, "ctx"))
    # kT ec0 chunk sc is consumed by scores at kt=4*sc
    for sc in range(1, NQH):
        _add(mids, 0, 4 * (sc - 1) + 1, lambda sc=sc: proj_qk(kT, "wk", 0, sc, "ctx"))
    for qh in range(1, NQH):  # qT ec0 for pair-0 blocks 1..3
        _add(mids, qh - 1, 13, lambda qh=qh: proj_qk(qT, "wq", 0, qh, "ctx"))
    for sc in range(NQH):  # kT ec1: one chunk per block 1..4 (needed from
        # block 4 = pair 1 qh 0, whose scores at kt=4*sc consume chunk sc)
        _add(mids, 1 + sc, 7, lambda sc=sc: proj_qk(kT, "wk", 1, sc, "ctx"))
    _add(mids, 3, 3, lambda: proj_qk(qT, "wq", 1, 0, "ctx"))
    for qh in range(1, NQH):  # qT ec1 for pair-1 blocks 1..3
        _add(mids, NQH + qh - 1, 13, lambda qh=qh: proj_qk(qT, "wq", 1, qh, "ctx"))

    fin_prev = None
    for b in range(2 * NQH):
        if fin_prev is not None:  # previous block's finalize halves, off
            _add(mids, b, 2, fin_prev[0])  # the boundary critical path
            _add(mids, b, 6, fin_prev[1])
        fin_prev = attn_block(b // NQH, b % NQH, mids[b], pres[b])
    fin_prev[0]()
    fin_prev[1]()


_COMPILED_NC = None


def _get_nc():
    global _COMPILED_NC
    if _COMPILED_NC is None:
        nc = bacc.Bacc(
            "TRN2", target_bir_lowering=False, debug=False, num_devices=N_CORES
        )
        with tile.TileContext(nc) as tc:
            with ExitStack() as ctx:
                _build_kernel(ctx, nc, tc)
        nc.compile()
        _COMPILED_NC = nc
    return _COMPILED_NC


def _shard_inputs(x, W_Q, W_K, W_V):
    """Per-core input maps: batch b = c//2, heads h0 = 4*(c%2)."""
    x = np.ascontiguousarray(np.asarray(x, dtype=np.float32))
    in_maps = []
    for c in range(N_CORES):
        b = c // 2
        h0 = HPC * (c % 2)
        rows = slice(h0 * DK, (h0 + HPC) * DK)
        in_maps.append(
            {
                "xt": np.ascontiguousarray(x[b].T),
                # torch Linear: y = x @ W.T -> project with W[rows].T [d, e]
                "wq": np.ascontiguousarray(np.asarray(W_Q)[rows].T.astype(np.float32)),
                "wk": np.ascontiguousarray(np.asarray(W_K)[rows].T.astype(np.float32)),
                "wv": np.ascontiguousarray(np.asarray(W_V)[rows].T.astype(np.float32)),
            }
        )
    return in_maps


def kernel(x, W_Q, W_K, W_V, _trace=False, _trace_kwargs=None):
    nc = _get_nc()
    in_maps = _shard_inputs(x, W_Q, W_K, W_V)
    res = run_bass_kernel_spmd(
        nc, in_maps, list(range(N_CORES)), trace=_trace, **(_trace_kwargs or {})
    )
    out = np.empty((B, S, D), dtype=np.float32)
    for c in range(N_CORES):
        b = c // 2
        h0 = HPC * (c % 2)
        out[b, :, h0 * DK : (h0 + HPC) * DK] = res.results[c]["out"]
    if _trace:
        return out, res
    return out


# revision 51
# speedup vs baseline: 1.0210x; 1.0210x over previous
"""Trainium2 Bass kernel for multi-head self-attention.

Problem: B=4, S=2048, D=512, H=8 heads (DK=64), no mask, softmax without
max-subtraction (faithful to reference): attn = exp(s) / (sum(exp(s)) + 1e-8).

Sharding over 8 cores: core c handles batch b = c // 2 and the 4 heads
h0 = 4*(c % 2) .. h0+4 (x sharded by batch, weights column-sharded by head).

Per-core device pipeline (all matmuls in bf16, fp32 accumulate):
  1. Load x_b [2048, 512] fp32, PE-transpose to xT [d, s], cast bf16.
  2. Project qT/kT [e, s] (e on partitions) and v [s, e] (natural, augmented
     with a ones column per head so the PV matmul also produces the softmax
     denominator row).
  3. Per (head, q-half): loop k-tiles: scoresT = kT.T @ qT in PSUM (fp32),
     ACT exp (scale=1/8 folded in) -> SBUF bf16, PV matmul accumulates
     ctx_aug [65, 1024] in PSUM (row 64 = denominator).
  4. Finalize: PE-transpose ctx back to [q, e], multiply by
     1/(denom + 1e-8), stage in SBUF, DMA out.
"""

import os
import sys
from contextlib import ExitStack

import numpy as np

# concourse ships with the container; make sure it is importable even if
# the caller's PYTHONPATH doesn't include the repo.
for _p in ("/opt/trn_rl_repo", "/opt/pypackages"):
    if os.path.isdir(_p) and _p not in sys.path:
        sys.path.append(_p)

import concourse.bass as bass
import concourse.tile as tile
from concourse import bacc, mybir
from concourse.bass_utils import run_bass_kernel_spmd
from concourse.masks import make_identity

F32 = mybir.dt.float32
BF16 = mybir.dt.bfloat16

B, S, D, H = 4, 2048, 512, 8
DK = D // H
SCALE = 1.0 / np.sqrt(DK)
N_CORES = 8
P = 128

HPC = H // 2          # heads per core = 4
E = HPC * DK          # per-core output width = 256
NS = S // P           # 16 s-tiles
NDC = D // P          # 4 d-chunks
NEC = E // P          # 2 e-chunks of projected heads
QH = 512              # q processed per attention block
NQH = S // QH         # 4
EA = DK + 1           # 65: head context + denominator row


def _build_kernel(ctx: ExitStack, nc: bass.Bass, tc: tile.TileContext):
    xt = nc.dram_tensor("xt", [D, S], F32, kind="ExternalInput").ap()
    wq = nc.dram_tensor("wq", [D, E], F32, kind="ExternalInput").ap()
    wk = nc.dram_tensor("wk", [D, E], F32, kind="ExternalInput").ap()
    wv = nc.dram_tensor("wv", [D, E], F32, kind="ExternalInput").ap()
    out = nc.dram_tensor("out", [S, E], F32, kind="ExternalOutput").ap()

    const = ctx.enter_context(tc.tile_pool(name="const", bufs=1))
    xstage = ctx.enter_context(tc.tile_pool(name="xstage", bufs=3))
    persist = ctx.enter_context(tc.tile_pool(name="persist", bufs=1))
    exps = ctx.enter_context(tc.tile_pool(name="exps", bufs=4))
    fin = ctx.enter_context(tc.tile_pool(name="fin", bufs=2))
    # PSUM budget (8 banks): "sc" slots 2 x 2 banks (f32 [128,1024] pair
    # scores, also proj/transpose staging) + "ctx" slots 4 x 1 bank
    # ([65,512] f32 PV accumulators, finalize transposes) = 8 banks.
    ps_big = ctx.enter_context(tc.tile_pool(name="ps_big", bufs=2, space="PSUM"))
    ps_ctx = ctx.enter_context(tc.tile_pool(name="ps_ctx", bufs=4, space="PSUM"))

    ident = const.tile([P, P], F32)
    make_identity(nc, ident)
    ident_bf = const.tile([P, P], BF16)
    nc.vector.tensor_copy(out=ident_bf[:], in_=ident[:])

    # ---- PE warmup: dense fp32 matmuls during the x DMA window so HAM
    # un-throttles (K=8/8) before the projection matmuls arrive ----
    # sized to span the whole x-chunk-0 DMA window: if the PE idles >3.4us
    # after the warmup, HAM re-throttles and the projections run at 1.2GHz
    warm = ps_big.tile([P, P], F32, tag="sc", name="warm")
    for _ in range(12):
        nc.tensor.matmul(warm[:], lhsT=ident, rhs=ident, start=True, stop=True)

    # ---- Phases A+B: x chunk 0 first (longest pole), weights threaded in,
    # then the remaining x chunks. x is transposed on the host; the DMA
    # still moves the full fp32 x. Cast everything to bf16 on DVE. ----
    xt3 = xt.rearrange("(dc p) s -> p dc s", p=P)
    xT = persist.tile([P, NDC, S], BF16, name="xT")
    w_bf = {}

    def load_x_chunk(sc, split=False):
        xs = xstage.tile([P, NDC, QH], F32, tag="xs", name="xs")
        if split:  # two DMA queues (HWDGE + SWDGE) halve the landing time
            nc.sync.dma_start(
                xs[:, :, 0 : QH // 2], xt3[:, :, sc * QH : sc * QH + QH // 2]
            )
            nc.gpsimd.dma_start(
                xs[:, :, QH // 2 : QH], xt3[:, :, sc * QH + QH // 2 : (sc + 1) * QH]
            )
        else:
            nc.sync.dma_start(xs[:], xt3[:, :, sc * QH : (sc + 1) * QH])
        nc.vector.tensor_copy(out=xT[:, :, sc * QH : (sc + 1) * QH], in_=xs[:])

    def load_w(name, wap):
        wf = xstage.tile([P, NDC, E], F32, tag="wstage", name=f"{name}f")
        nc.sync.dma_start(wf[:], wap.rearrange("(dc p) e -> p dc e", p=P))
        wb = persist.tile([P, NDC, E], BF16, tag=f"{name}b", name=f"{name}b")
        nc.vector.tensor_copy(out=wb[:], in_=wf[:])
        w_bf[name] = wb

    load_x_chunk(0)
    load_w("wk", wk)
    load_w("wq", wq)
    load_w("wv", wv)
    load_x_chunk(1)
    load_x_chunk(2)
    load_x_chunk(3)

    # qT/kT: [e, s] with e on partitions (2 chunks of 128 = 4 heads)
    qT = persist.tile([P, NEC, S], BF16, name="qT")
    kT = persist.tile([P, NEC, S], BF16, name="kT")
    # v natural [s, e] in ones-augmented per-head layout [128, h, 65]
    v_aug = persist.tile([P, NS, HPC, EA], BF16, name="v_aug")
    nc.gpsimd.memset(v_aug[:, :, :, DK], 1.0)
    out_sb = persist.tile([P, NS, E], F32, name="out_sb")

    def proj_qk(dst, wname, ec, sc, tag, s0=0, sw=512):
        """dst[:, ec, sc*512+s0 : +sw] = (W row chunk).T @ xT chunk (bf16)."""
        wb = w_bf[wname]
        pp = ps_big.tile([P, 512], F32, tag=tag, name="pp") if tag == "sc" else \
            ps_ctx.tile([P, 512], F32, tag=tag, name="pp")
        lo = sc * 512 + s0
        for dc in range(NDC):
            nc.tensor.matmul(
                pp[:, 0:sw],
                lhsT=wb[:, dc, ec * P : (ec + 1) * P],
                rhs=xT[:, dc, lo : lo + sw],
                start=(dc == 0),
                stop=(dc == NDC - 1),
            )
        nc.vector.tensor_copy(out=dst[:, ec, lo : lo + sw], in_=pp[:, 0:sw])

    def proj_v(st, tag):
        vp = ps_big.tile([P, E], F32, tag=tag, name="vp") if tag == "sc" else \
            ps_ctx.tile([P, E], F32, tag=tag, name="vp")
        wvb = w_bf["wv"]
        for dc in range(NDC):
            nc.tensor.matmul(
                vp[:],
                lhsT=xT[:, dc, st * P : (st + 1) * P],
                rhs=wvb[:, dc, :],
                start=(dc == 0),
                stop=(dc == NDC - 1),
            )
        nc.vector.tensor_copy(
            out=v_aug[:, st, :, 0:DK],
            in_=vp.rearrange("p (h e) -> p h e", e=DK),
        )

    def attn_block(ec, qh, mid_work=None, pre_pv_work=None):
        """One (head pair, q-chunk) attention block. mid_work / pre_pv_work
        map kt -> thunks emitted inside the k loop (after PV / between exp
        and PV) so projections hide in the PE slack of the ACT stream."""
        ctx_a = ps_ctx.tile([EA, QH], F32, tag="ctx", name="ctx_a")
        ctx_b = ps_ctx.tile([EA, QH], F32, tag="ctx", name="ctx_b")
        for kt in range(NS):
            # scores for both heads, f32 psum [128, 2*QH]; the two matmuls
            # occupy PE row groups 0-63 / 64-127 concurrently
            sc_ps = ps_big.tile([P, 2 * QH], F32, tag="sc", name="sc_ps")
            for hb in range(2):
                nc.tensor.matmul(
                    sc_ps[:, hb * QH : (hb + 1) * QH],
                    lhsT=kT[hb * DK : (hb + 1) * DK, ec, kt * P : (kt + 1) * P],
                    rhs=qT[hb * DK : (hb + 1) * DK, ec, qh * QH : (qh + 1) * QH],
                    start=True,
                    stop=True,
                )
            ex = exps.tile([P, 2 * QH], BF16, tag="ex", name="ex")
            nc.scalar.activation(
                ex[:], sc_ps[:], mybir.ActivationFunctionType.Exp, scale=SCALE
            )
            if pre_pv_work and kt in pre_pv_work:
                for thunk in pre_pv_work[kt]:
                    thunk()
            for hb, ctx_ps in ((0, ctx_a), (1, ctx_b)):
                nc.tensor.matmul(
                    ctx_ps[:],
                    lhsT=v_aug[:, kt, 2 * ec + hb, :],
                    rhs=ex[:, hb * QH : (hb + 1) * QH],
                    start=(kt == 0),
                    stop=(kt == NS - 1),
                )
            if mid_work and kt in mid_work:
                for thunk in mid_work[kt]:
                    thunk()

        # finalize, split per head so the two halves land in different kts
        # of the next block: transpose ctx back to [q, e] (bf16 input
        # halves the PE cost), normalize into a shared staging tile, DMA
        # out after the second head
        po_cell = []

        def fin_head(hb, ctx_ps):
            if not po_cell:
                po_cell.append(fin.tile([P, QH // P, P], F32, tag="po", name="po"))
            po = po_cell[0]
            caug = fin.tile([EA, QH], BF16, tag="caug", name="caug")
            nc.vector.tensor_copy(out=caug[:], in_=ctx_ps[:])
            pt = ps_ctx.tile([P, 4, EA + 1], BF16, tag="ctx", name="pt")
            for j in range(QH // P):
                nc.tensor.transpose(
                    pt[:, j, 0:EA],
                    caug[:, j * P : (j + 1) * P],
                    ident_bf[0:EA, 0:EA],
                )
            den = fin.tile([P, 4], F32, tag="den", name="den")
            nc.vector.tensor_scalar_add(den[:], pt[:, :, DK], 1e-8)
            rec = fin.tile([P, 4], F32, tag="rec", name="rec")
            nc.vector.reciprocal(rec[:], den[:])
            for j in range(QH // P):
                nc.vector.tensor_scalar_mul(
                    po[:, j, hb * DK : (hb + 1) * DK],
                    pt[:, j, 0:DK],
                    rec[:, j : j + 1],
                )
            if hb == 1:
                nc.sync.dma_start(
                    out[qh * QH : (qh + 1) * QH, ec * P : (ec + 1) * P].rearrange(
                        "(t p) e -> p t e", p=P
                    ),
                    po[:],
                )

        return (lambda: fin_head(0, ctx_a), lambda: fin_head(1, ctx_b))

    # ---- Phases C+D interleaved: only block 0's immediate dependencies
    # are projected up front; everything else is paced into the k loops of
    # the 8 attention blocks to hide in PE slack under the ACT stream ----
    # first scores matmul only reads kT columns 0-127, so project those
    # first to start the exp stream sooner; the rest lands in block 0's
    # first pre-PV slot (behind scores kt0, ahead of scores kt1)
    proj_qk(kT, "wk", 0, 0, "sc", s0=0, sw=P)
    proj_qk(qT, "wq", 0, 0, "sc")

    mids = [dict() for _ in range(2 * NQH)]
    pres = [dict() for _ in range(2 * NQH)]

    def _add(d, b, kt, thunk):
        d[b].setdefault(kt, []).append(thunk)

    _add(pres, 0, 0, lambda: proj_qk(kT, "wk", 0, 0, "ctx", s0=P, sw=512 - P))
    # v projections: v[kt] is consumed by block 0's PV at kt, so emit
    # proj_v(kt+1) between exp(kt) and PV(kt); v0 also goes there (off the
    # first-scores critical path -- it would otherwise gate the exp stream
    # on the wv DMA)
    _add(pres, 0, 0, lambda: proj_v(0, "ctx"))
    for st in range(1, NS):
        _add(pres, 0, st - 1, lambda st=st: proj_v(st, "ctx"))
    # kT ec0 chunk sc is consumed by scores at kt=4*sc
    for sc in range(1, NQH):
        _add(mids, 0, 4 * (sc - 1) + 1, lambda sc=sc: proj_qk(kT, "wk", 0, sc, "ctx"))
    for qh in range(1, NQH):  # qT ec0 for pair-0 blocks 1..3
        _add(mids, qh - 1, 13, lambda qh=qh: proj_qk(qT, "wq", 0, qh, "ctx"))
    for sc in range(NQH):  # kT ec1: one chunk per block 1..4 (needed from
        # block 4 = pair 1 qh 0, whose scores at kt=4*sc consume chunk sc)
        _add(mids, 1 + sc, 7, lambda sc=sc: proj_qk(kT, "wk", 1, sc, "ctx"))
    _add(mids, 3, 3, lambda: proj_qk(qT, "wq", 1, 0, "ctx"))
    for qh in range(1, NQH):  # qT ec1 for pair-1 blocks 1..3
        _add(mids, NQH + qh - 1, 13, lambda qh=qh: proj_qk(qT, "wq", 1, qh, "ctx"))

    fin_prev = None
    for b in range(2 * NQH):
        if fin_prev is not None:  # previous block's finalize halves, off
            _add(mids, b, 2, fin_prev[0])  # the boundary critical path
            _add(mids, b, 6, fin_prev[1])
        fin_prev = attn_block(b // NQH, b % NQH, mids[b], pres[b])
    fin_prev[0]()
    fin_prev[1]()


_COMPILED_NC = None


def _get_nc():
    global _COMPILED_NC
    if _COMPILED_NC is None:
        nc = bacc.Bacc(
            "TRN2", target_bir_lowering=False, debug=False, num_devices=N_CORES
        )
        with tile.TileContext(nc) as tc:
            with ExitStack() as ctx:
                _build_kernel(ctx, nc, tc)
        nc.compile()
        _COMPILED_NC = nc
    return _COMPILED_NC


def _shard_inputs(x, W_Q, W_K, W_V):
    """Per-core input maps: batch b = c//2, heads h0 = 4*(c%2)."""
    x = np.ascontiguousarray(np.asarray(x, dtype=np.float32))
    in_maps = []
    for c in range(N_CORES):
        b = c // 2
        h0 = HPC * (c % 2)
        rows = slice(h0 * DK, (h0 + HPC) * DK)
        in_maps.append(
            {
                "xt": np.ascontiguousarray(x[b].T),
                # torch Linear: y = x @ W.T -> project with W[rows].T [d, e]
                "wq": np.ascontiguousarray(np.asarray(W_Q)[rows].T.astype(np.float32)),
                "wk": np.ascontiguousarray(np.asarray(W_K)[rows].T.astype(np.float32)),
                "wv": np.ascontiguousarray(np.asarray(W_V)[rows].T.astype(np.float32)),
            }
        )
    return in_maps


def kernel(x, W_Q, W_K, W_V, _trace=False, _trace_kwargs=None):
    nc = _get_nc()
    in_maps = _shard_inputs(x, W_Q, W_K, W_V)
    res = run_bass_kernel_spmd(
        nc, in_maps, list(range(N_CORES)), trace=_trace, **(_trace_kwargs or {})
    )
    out = np.empty((B, S, D), dtype=np.float32)
    for c in range(N_CORES):
        b = c // 2
        h0 = HPC * (c % 2)
        out[b, :, h0 * DK : (h0 + HPC) * DK] = res.results[c]["out"]
    if _trace:
        return out, res
    return out
